# revision 23
# baseline (speedup 1.0000x reference)
"""BlockSparseLinear kernel for Trainium2 (8 NeuronCores, Bass/Tile).

Computes y = x @ W.T + bias with x [8192, 4096] fp32, W [4096, 4096] fp32
(block-masked; treated densely — the 16x16 block granularity is finer than
the PE's 128-deep contraction and the pattern is unstructured, so dense
matmul is the compute roofline), bias [4096].

Numerics: x and W are cast to bf16 on the host (exact rel err vs fp32
reference measured at 2.3e-3, well inside the 2e-2 gate). bf16 matmuls
run 1 cycle/row on the PE (measured 215-216ns per 128x128x512 matmul =
~2.37 GHz sustained) vs fp32r's 227ns, and halve x/W DMA traffic.
PSUM accumulation and the bias epilogue stay fp32.

Sharding: 8-way data-parallel over tokens. Each core computes
yT_c = W @ xT_c + bias for its 1024-token slice.

Per-core kernel (yT layout, outputs on PSUM partitions):
  out[oi=128, t=512] += wT_tile[k=128, oi=128].T @ xT_tile[k=128, t=512]
  - x shard (8.4 MB bf16) resident in SBUF; W streamed column-by-column.
  - bias fused into the PSUM->SBUF eviction on VectorE.
  - x loads issue on the Scalar (Activation) HWDGE queue, w/bias/out on
    the Sync queue: two queues in parallel shorten the critical path to
    the first matmul and keep the PE fed during the DVFS ramp.
  - last output column runs its two t-halves serially so the first
    half's eviction+store hides under the second half's matmuls.

Host side packs inputs so every DMA is contiguous per partition:
  xt[c, p, ko, t] = x[c*1024+t, ko*128+p]          (bf16)
  wp[oc, p, ko, oi] = W[oc*128+oi, ko*128+p]       (bf16, = W.T tiles)
  bs[p, oc] = bias[oc*128+p]                       (fp32)
  output yt[oc, p, t] = y[c*1024+t, oc*128+p]      (fp32)
"""

import os

import numpy as np

N_CORES = 8
TOK = 8192
T_PER_CORE = TOK // N_CORES  # 1024
D_IN = 4096
D_OUT = 4096
P = 128
KO = D_IN // P  # 32 contraction tiles
OC = D_OUT // P  # 32 output column tiles
T_FREE = 512  # moving free dim per matmul
NT = T_PER_CORE // T_FREE  # 2

LAST_EXEC_NS = None

_cache = {}


def _build_bass():
    import concourse.bacc as bacc
    import concourse.mybir as mybir
    import concourse.tile as tile

    f32 = mybir.dt.float32
    bf16 = mybir.dt.bfloat16

    nc = bacc.Bacc(
        "TRN2",
        target_bir_lowering=False,
        debug=False,
        num_devices=N_CORES,
        name="block_sparse_linear",
        dynamic_dma_scratch_size=4096,
    )

    xt = nc.dram_tensor("xt", [P, KO, T_PER_CORE], bf16, kind="ExternalInput")
    wp = nc.dram_tensor("wp", [OC, P, KO, P], bf16, kind="ExternalInput")
    bs = nc.dram_tensor("bs", [P, OC], f32, kind="ExternalInput")
    yt = nc.dram_tensor("yt", [OC, P, T_PER_CORE], f32, kind="ExternalOutput")

    WAVE = 4  # leading output columns processed ko-interleaved during x load

    with tile.TileContext(nc) as tc:
        with (
            tc.tile_pool(name="xpool", bufs=1) as xpool,
            tc.tile_pool(name="wpool", bufs=WAVE + 1) as wpool,
            tc.tile_pool(name="opool", bufs=4) as opool,
            tc.tile_pool(name="bpool", bufs=1) as bpool,
            tc.tile_pool(name="pspool", bufs=8, space="PSUM") as pspool,
        ):
            # Resident x shard; per (ko, t-half) pieces so ramp matmuls can
            # start as soon as each 128KB piece lands. x rides the Scalar
            # HWDGE queue, w rides Sync: the two streams never serialize
            # behind each other at issue time.
            x_sb = xpool.tile([P, KO, T_PER_CORE], bf16)
            w_wave = [
                wpool.tile([P, KO, P], bf16, tag="w", name=f"w_{oc}")
                for oc in range(WAVE)
            ]

            def dma_x(ko, t):
                nc.scalar.dma_start(
                    x_sb[:, ko, t * T_FREE : (t + 1) * T_FREE],
                    xt[:, ko, t * T_FREE : (t + 1) * T_FREE],
                )

            def dma_w(w_sb, oc, k0, k1):
                nc.sync.dma_start(w_sb[:, k0:k1, :], wp[oc, :, k0:k1, :])

            # Critical path first: the (ko0,t0) matmul needs x(ko0,t0) and
            # w col0 ko0 — issue those two immediately, smallest first, on
            # different queues (x rides Scalar's HWDGE queue, w Sync's, so
            # the streams never serialize behind each other in a FIFO).
            dma_w(w_wave[0], 0, 0, 2)  # 64KB
            dma_x(0, 0)  # 128KB, parallel queue
            for c in range(1, WAVE):
                dma_w(w_wave[c], c, 0, 2)
            dma_x(0, 1)
            dma_x(1, 0)
            dma_x(1, 1)
            for c in range(WAVE):
                dma_w(w_wave[c], c, 2, 8)  # 192KB each
            dma_x(2, 0)
            dma_x(2, 1)
            bias_sb = bpool.tile([P, OC], f32)
            nc.sync.dma_start(bias_sb[:], bs[:])
            # remaining x (ko 3..31) on scalar; remaining wave w
            # ([8:16],[16:24],[24:32] per col) on sync. Per-DMA queue
            # overhead (~0.5us kickoff) is what starves the ramp, not
            # engine bandwidth — so x rides in 2-ko 512KB chunks: 15
            # chunks x ~2us queue occupancy fits the ~55us ramp window
            # with margin.
            dma_x(3, 0)
            dma_x(3, 1)
            dma_x(4, 0)
            dma_x(4, 1)
            dma_x(5, 0)
            dma_x(5, 1)
            w_rest = [(c, k0, k0 + 8) for k0 in (8, 16, 24) for c in range(WAVE)]
            wi = 0
            for ko in range(6, KO, 2):
                nc.scalar.dma_start(
                    x_sb[:, ko : ko + 2, :], xt[:, ko : ko + 2, :]
                )
                # one w chunk (256KB) per x pair keeps w ~a column ahead
                # of its ko deadline without starving x.
                if wi < len(w_rest):
                    c, k0, k1 = w_rest[wi]
                    dma_w(w_wave[c], c, k0, k1)
                    wi += 1
            while wi < len(w_rest):
                c, k0, k1 = w_rest[wi]
                dma_w(w_wave[c], c, k0, k1)
                wi += 1

            def evict(oc, ps, t):
                o_sb = opool.tile([P, T_FREE], f32, tag="o", name=f"o_{oc}_{t}")
                # out = psum + bias[p] on VectorE (free-dim-broadcast bias).
                nc.vector.tensor_tensor(
                    o_sb[:],
                    ps[:],
                    bias_sb[:, oc : oc + 1].to_broadcast([P, T_FREE]),
                    mybir.AluOpType.add,
                )
                # output stores ride the Scalar queue; the Sync queue
                # carries the dense-phase w stream.
                nc.scalar.dma_start(
                    yt[oc, :, t * T_FREE : (t + 1) * T_FREE], o_sb[:]
                )

            # Ramp phase: first WAVE output columns interleaved by ko, so
            # every arriving x piece enables WAVE matmuls.
            ps_wave = [
                [
                    pspool.tile([P, T_FREE], f32, tag="ps", name=f"ps_{oc}_{t}")
                    for t in range(NT)
                ]
                for oc in range(WAVE)
            ]
            # DVFS warmup: the PE clock ramps 0.65 -> 1.2 -> 2.4 GHz after
            # ~3us of continuous busy time. Burn dummy matmuls (zeroed
            # operands, results wiped by the real ko0 start=True reset)
            # during the ~4.5us DMA lead-in so the real matmuls start at
            # full clock instead of paying ~10us of slow-ramp excess.
            # They write the wave tile whose first real use comes latest
            # (col WAVE-1, t1), so the warmup chain never delays real work.
            # Warmup operands come from the runtime-reserved dynamic-DMA
            # scratch (always allocated, contents irrelevant): zero
            # dependencies, so the chain starts the moment the PE queue
            # clears its preamble (~5.8us) instead of waiting on a memset.
            scratch = nc.dma_scratch[:, :2048].bitcast(bf16)
            for _ in range(7):
                nc.tensor.matmul(
                    ps_wave[WAVE - 1][NT - 1][:],
                    scratch[:, :P],
                    scratch[:, P : P + T_FREE],
                    start=True,
                    stop=True,
                    skip_group_check=True,
                )
            for ko in range(KO):
                for t in range(NT):
                    for oc in range(WAVE):
                        nc.tensor.matmul(
                            ps_wave[oc][t][:],
                            w_wave[oc][:, ko, :],
                            x_sb[:, ko, t * T_FREE : (t + 1) * T_FREE],
                            start=(ko == 0),
                            stop=(ko == KO - 1),
                        )
            for oc in range(WAVE):
                for t in range(NT):
                    evict(oc, ps_wave[oc][t], t)

            # Dense phase: x resident; stream one w column per output
            # column. Last column runs t-serial so its first half's
            # eviction+store hides under the second half's matmuls.
            for oc in range(WAVE, OC):
                w_sb = wpool.tile([P, KO, P], bf16, tag="w", name=f"w_{oc}")
                for k0 in range(0, KO, 8):
                    dma_w(w_sb, oc, k0, k0 + 8)
                ps_t = [
                    pspool.tile([P, T_FREE], f32, tag="ps", name=f"ps_{oc}_{t}")
                    for t in range(NT)
                ]
                if oc == OC - 1:
                    # Final column: quarter-width (N=256) serial passes so
                    # each quarter's eviction+store hides under the next
                    # quarter's matmuls; only the last quarter's epilogue
                    # (~0.6us) is exposed at the kernel tail.
                    QW = 256
                    for q in range(T_PER_CORE // QW):
                        t, qo = divmod(q * QW, T_FREE)
                        ps_q = ps_t[t][:, qo : qo + QW]
                        for ko in range(KO):
                            nc.tensor.matmul(
                                ps_q,
                                w_sb[:, ko, :],
                                x_sb[:, ko, q * QW : (q + 1) * QW],
                                start=(ko == 0),
                                stop=(ko == KO - 1),
                            )
                        o_sb = opool.tile([P, QW], f32, tag="o", name=f"oq_{q}")
                        nc.vector.tensor_tensor(
                            o_sb[:],
                            ps_q,
                            bias_sb[:, oc : oc + 1].to_broadcast([P, QW]),
                            mybir.AluOpType.add,
                        )
                        nc.scalar.dma_start(
                            yt[oc, :, q * QW : (q + 1) * QW], o_sb[:]
                        )
                else:
                    for ko in range(KO):
                        for t in range(NT):
                            nc.tensor.matmul(
                                ps_t[t][:],
                                w_sb[:, ko, :],
                                x_sb[:, ko, t * T_FREE : (t + 1) * T_FREE],
                                start=(ko == 0),
                                stop=(ko == KO - 1),
                            )
                    for t in range(NT):
                        evict(oc, ps_t[t], t)

    nc.compile()
    return nc


def _pack_inputs(x, weight, bias):
    import ml_dtypes

    bf16 = ml_dtypes.bfloat16
    x = np.ascontiguousarray(x, dtype=np.float32)
    weight = np.ascontiguousarray(weight, dtype=np.float32)
    bias = np.ascontiguousarray(bias, dtype=np.float32)

    # xt[c, p, ko, t] = x[c*T + t, ko*P + p]
    xt = np.ascontiguousarray(
        x.reshape(N_CORES, T_PER_CORE, KO, P).transpose(0, 3, 2, 1).astype(bf16)
    )
    # wp[oc, p, ko, oi] = W[oc*P + oi, ko*P + p]
    wp = np.ascontiguousarray(
        weight.reshape(OC, P, KO, P).transpose(0, 3, 2, 1).astype(bf16)
    )
    # bs[p, oc] = bias[oc*P + p]
    bs = np.ascontiguousarray(bias.reshape(OC, P).T)
    return xt, wp, bs


def kernel(x, weight, bias):
    global LAST_EXEC_NS
    from concourse import bass_utils

    if "nc" not in _cache:
        _cache["nc"] = _build_bass()
    nc = _cache["nc"]

    xt, wp, bs = _pack_inputs(x, weight, bias)

    in_maps = [{"xt": xt[c], "wp": wp, "bs": bs} for c in range(N_CORES)]

    trace = bool(int(os.environ.get("BSL_TRACE", "0")))
    res = bass_utils.run_bass_kernel_spmd(
        nc,
        in_maps,
        core_ids=list(range(N_CORES)),
        trace=trace,
    )
    LAST_EXEC_NS = res.exec_time_ns
    _cache["last_res"] = res

    # yt[c][oc, p, t] -> y[c*T + t, oc*P + p]
    out = np.empty((TOK, D_OUT), dtype=np.float32)
    for c in range(N_CORES):
        yt = res.results[c]["yt"]
        out[c * T_PER_CORE : (c + 1) * T_PER_CORE] = (
            yt.transpose(2, 0, 1).reshape(T_PER_CORE, D_OUT)
        )
    return out


# revision 24
# speedup vs baseline: 1.0009x; 1.0009x over previous
"""BlockSparseLinear kernel for Trainium2 (8 NeuronCores, Bass/Tile).

Computes y = x @ W.T + bias with x [8192, 4096] fp32, W [4096, 4096] fp32
(block-masked; treated densely — the 16x16 block granularity is finer than
the PE's 128-deep contraction and the pattern is unstructured, so dense
matmul is the compute roofline), bias [4096].

Numerics: x and W are cast to bf16 on the host (exact rel err vs fp32
reference measured at 2.3e-3, well inside the 2e-2 gate). bf16 matmuls
run 1 cycle/row on the PE (measured 215-216ns per 128x128x512 matmul =
~2.37 GHz sustained) vs fp32r's 227ns, and halve x/W DMA traffic.
PSUM accumulation and the bias epilogue stay fp32.

Sharding: 8-way data-parallel over tokens. Each core computes
yT_c = W @ xT_c + bias for its 1024-token slice.

Per-core kernel (yT layout, outputs on PSUM partitions):
  out[oi=128, t=512] += wT_tile[k=128, oi=128].T @ xT_tile[k=128, t=512]
  - x shard (8.4 MB bf16) resident in SBUF; W streamed column-by-column.
  - bias fused into the PSUM->SBUF eviction on VectorE.
  - x loads issue on the Scalar (Activation) HWDGE queue, w/bias/out on
    the Sync queue: two queues in parallel shorten the critical path to
    the first matmul and keep the PE fed during the DVFS ramp.
  - last output column runs its two t-halves serially so the first
    half's eviction+store hides under the second half's matmuls.

Host side packs inputs so every DMA is contiguous per partition:
  xt[c, p, ko, t] = x[c*1024+t, ko*128+p]          (bf16)
  wp[oc, p, ko, oi] = W[oc*128+oi, ko*128+p]       (bf16, = W.T tiles)
  bs[p, oc] = bias[oc*128+p]                       (fp32)
  output yt[oc, p, t] = y[c*1024+t, oc*128+p]      (fp32)
"""

import os

import numpy as np

N_CORES = 8
TOK = 8192
T_PER_CORE = TOK // N_CORES  # 1024
D_IN = 4096
D_OUT = 4096
P = 128
KO = D_IN // P  # 32 contraction tiles
OC = D_OUT // P  # 32 output column tiles
T_FREE = 512  # moving free dim per matmul
NT = T_PER_CORE // T_FREE  # 2

LAST_EXEC_NS = None

_cache = {}


def _build_bass():
    import concourse.bacc as bacc
    import concourse.mybir as mybir
    import concourse.tile as tile

    f32 = mybir.dt.float32
    bf16 = mybir.dt.bfloat16

    nc = bacc.Bacc(
        "TRN2",
        target_bir_lowering=False,
        debug=False,
        num_devices=N_CORES,
        name="block_sparse_linear",
        dynamic_dma_scratch_size=4096,
    )

    xt = nc.dram_tensor("xt", [P, KO, T_PER_CORE], bf16, kind="ExternalInput")
    wp = nc.dram_tensor("wp", [OC, P, KO, P], bf16, kind="ExternalInput")
    bs = nc.dram_tensor("bs", [P, OC], f32, kind="ExternalInput")
    yt = nc.dram_tensor("yt", [OC, P, T_PER_CORE], f32, kind="ExternalOutput")

    WAVE = 4  # leading output columns processed ko-interleaved during x load

    with tile.TileContext(nc) as tc:
        with (
            tc.tile_pool(name="xpool", bufs=1) as xpool,
            tc.tile_pool(name="wpool", bufs=WAVE + 1) as wpool,
            tc.tile_pool(name="opool", bufs=4) as opool,
            tc.tile_pool(name="bpool", bufs=1) as bpool,
            tc.tile_pool(name="pspool", bufs=8, space="PSUM") as pspool,
        ):
            # Resident x shard; per (ko, t-half) pieces so ramp matmuls can
            # start as soon as each 128KB piece lands. x rides the Scalar
            # HWDGE queue, w rides Sync: the two streams never serialize
            # behind each other at issue time.
            x_sb = xpool.tile([P, KO, T_PER_CORE], bf16)
            w_wave = [
                wpool.tile([P, KO, P], bf16, tag="w", name=f"w_{oc}")
                for oc in range(WAVE)
            ]

            def dma_x(ko, t):
                nc.scalar.dma_start(
                    x_sb[:, ko, t * T_FREE : (t + 1) * T_FREE],
                    xt[:, ko, t * T_FREE : (t + 1) * T_FREE],
                )

            def dma_w(w_sb, oc, k0, k1):
                nc.sync.dma_start(w_sb[:, k0:k1, :], wp[oc, :, k0:k1, :])

            # Critical path first: the (ko0,t0) matmul needs x(ko0,t0) and
            # w col0 ko0 — issue those two immediately, smallest first, on
            # different queues (x rides Scalar's HWDGE queue, w Sync's, so
            # the streams never serialize behind each other in a FIFO).
            dma_w(w_wave[0], 0, 0, 2)  # 64KB
            dma_x(0, 0)  # 128KB, parallel queue
            for c in range(1, WAVE):
                dma_w(w_wave[c], c, 0, 2)
            dma_x(0, 1)
            dma_x(1, 0)
            dma_x(1, 1)
            for c in range(WAVE):
                dma_w(w_wave[c], c, 2, 8)  # 192KB each
            dma_x(2, 0)
            dma_x(2, 1)
            bias_sb = bpool.tile([P, OC], f32)
            nc.sync.dma_start(bias_sb[:], bs[:])
            # remaining x (ko 3..31) on scalar; remaining wave w
            # ([8:16],[16:24],[24:32] per col) on sync. Per-DMA queue
            # overhead (~0.5us kickoff) is what starves the ramp, not
            # engine bandwidth — so x rides in 2-ko 512KB chunks: 15
            # chunks x ~2us queue occupancy fits the ~55us ramp window
            # with margin.
            dma_x(3, 0)
            dma_x(3, 1)
            dma_x(4, 0)
            dma_x(4, 1)
            dma_x(5, 0)
            dma_x(5, 1)
            w_rest = [(c, k0, k0 + 8) for k0 in (8, 16, 24) for c in range(WAVE)]
            wi = 0
            for ko in range(6, KO, 2):
                nc.scalar.dma_start(
                    x_sb[:, ko : ko + 2, :], xt[:, ko : ko + 2, :]
                )
                # one w chunk (256KB) per x pair keeps w ~a column ahead
                # of its ko deadline without starving x.
                if wi < len(w_rest):
                    c, k0, k1 = w_rest[wi]
                    dma_w(w_wave[c], c, k0, k1)
                    wi += 1
            while wi < len(w_rest):
                c, k0, k1 = w_rest[wi]
                dma_w(w_wave[c], c, k0, k1)
                wi += 1

            def evict(oc, ps, t):
                o_sb = opool.tile([P, T_FREE], f32, tag="o", name=f"o_{oc}_{t}")
                # out = psum + bias[p] on VectorE (free-dim-broadcast bias).
                nc.vector.tensor_tensor(
                    o_sb[:],
                    ps[:],
                    bias_sb[:, oc : oc + 1].to_broadcast([P, T_FREE]),
                    mybir.AluOpType.add,
                )
                # output stores ride the Scalar queue; the Sync queue
                # carries the dense-phase w stream.
                nc.scalar.dma_start(
                    yt[oc, :, t * T_FREE : (t + 1) * T_FREE], o_sb[:]
                )

            # Ramp phase: first WAVE output columns interleaved by ko, so
            # every arriving x piece enables WAVE matmuls.
            ps_wave = [
                [
                    pspool.tile([P, T_FREE], f32, tag="ps", name=f"ps_{oc}_{t}")
                    for t in range(NT)
                ]
                for oc in range(WAVE)
            ]
            # DVFS warmup: the PE clock ramps 0.65 -> 1.2 -> 2.4 GHz after
            # ~3us of continuous busy time. Burn dummy matmuls (zeroed
            # operands, results wiped by the real ko0 start=True reset)
            # during the ~4.5us DMA lead-in so the real matmuls start at
            # full clock instead of paying ~10us of slow-ramp excess.
            # They write the wave tile whose first real use comes latest
            # (col WAVE-1, t1), so the warmup chain never delays real work.
            # Warmup operands come from the runtime-reserved dynamic-DMA
            # scratch (always allocated, contents irrelevant): zero
            # dependencies, so the chain starts the moment the PE queue
            # clears its preamble (~5.8us) instead of waiting on a memset.
            scratch = nc.dma_scratch[:, :2048].bitcast(bf16)
            for _ in range(10):
                nc.tensor.matmul(
                    ps_wave[WAVE - 1][NT - 1][:],
                    scratch[:, :P],
                    scratch[:, P : P + T_FREE],
                    start=True,
                    stop=True,
                    skip_group_check=True,
                )
            for ko in range(KO):
                for t in range(NT):
                    for oc in range(WAVE):
                        nc.tensor.matmul(
                            ps_wave[oc][t][:],
                            w_wave[oc][:, ko, :],
                            x_sb[:, ko, t * T_FREE : (t + 1) * T_FREE],
                            start=(ko == 0),
                            stop=(ko == KO - 1),
                        )
            for oc in range(WAVE):
                for t in range(NT):
                    evict(oc, ps_wave[oc][t], t)

            # Dense phase: x resident; stream one w column per output
            # column. Last column runs t-serial so its first half's
            # eviction+store hides under the second half's matmuls.
            for oc in range(WAVE, OC):
                w_sb = wpool.tile([P, KO, P], bf16, tag="w", name=f"w_{oc}")
                for k0 in range(0, KO, 8):
                    dma_w(w_sb, oc, k0, k0 + 8)
                ps_t = [
                    pspool.tile([P, T_FREE], f32, tag="ps", name=f"ps_{oc}_{t}")
                    for t in range(NT)
                ]
                if oc == OC - 1:
                    # Final column: quarter-width (N=256) serial passes so
                    # each quarter's eviction+store hides under the next
                    # quarter's matmuls; only the last quarter's epilogue
                    # (~0.6us) is exposed at the kernel tail.
                    QW = 256
                    for q in range(T_PER_CORE // QW):
                        t, qo = divmod(q * QW, T_FREE)
                        ps_q = ps_t[t][:, qo : qo + QW]
                        for ko in range(KO):
                            nc.tensor.matmul(
                                ps_q,
                                w_sb[:, ko, :],
                                x_sb[:, ko, q * QW : (q + 1) * QW],
                                start=(ko == 0),
                                stop=(ko == KO - 1),
                            )
                        o_sb = opool.tile([P, QW], f32, tag="o", name=f"oq_{q}")
                        nc.vector.tensor_tensor(
                            o_sb[:],
                            ps_q,
                            bias_sb[:, oc : oc + 1].to_broadcast([P, QW]),
                            mybir.AluOpType.add,
                        )
                        nc.scalar.dma_start(
                            yt[oc, :, q * QW : (q + 1) * QW], o_sb[:]
                        )
                else:
                    for ko in range(KO):
                        for t in range(NT):
                            nc.tensor.matmul(
                                ps_t[t][:],
                                w_sb[:, ko, :],
                                x_sb[:, ko, t * T_FREE : (t + 1) * T_FREE],
                                start=(ko == 0),
                                stop=(ko == KO - 1),
                            )
                    for t in range(NT):
                        evict(oc, ps_t[t], t)

    nc.compile()
    return nc


def _pack_inputs(x, weight, bias):
    import ml_dtypes

    bf16 = ml_dtypes.bfloat16
    x = np.ascontiguousarray(x, dtype=np.float32)
    weight = np.ascontiguousarray(weight, dtype=np.float32)
    bias = np.ascontiguousarray(bias, dtype=np.float32)

    # xt[c, p, ko, t] = x[c*T + t, ko*P + p]
    xt = np.ascontiguousarray(
        x.reshape(N_CORES, T_PER_CORE, KO, P).transpose(0, 3, 2, 1).astype(bf16)
    )
    # wp[oc, p, ko, oi] = W[oc*P + oi, ko*P + p]
    wp = np.ascontiguousarray(
        weight.reshape(OC, P, KO, P).transpose(0, 3, 2, 1).astype(bf16)
    )
    # bs[p, oc] = bias[oc*P + p]
    bs = np.ascontiguousarray(bias.reshape(OC, P).T)
    return xt, wp, bs


def kernel(x, weight, bias):
    global LAST_EXEC_NS
    from concourse import bass_utils

    if "nc" not in _cache:
        _cache["nc"] = _build_bass()
    nc = _cache["nc"]

    xt, wp, bs = _pack_inputs(x, weight, bias)

    in_maps = [{"xt": xt[c], "wp": wp, "bs": bs} for c in range(N_CORES)]

    trace = bool(int(os.environ.get("BSL_TRACE", "0")))
    res = bass_utils.run_bass_kernel_spmd(
        nc,
        in_maps,
        core_ids=list(range(N_CORES)),
        trace=trace,
    )
    LAST_EXEC_NS = res.exec_time_ns
    _cache["last_res"] = res

    # yt[c][oc, p, t] -> y[c*T + t, oc*P + p]
    out = np.empty((TOK, D_OUT), dtype=np.float32)
    for c in range(N_CORES):
        yt = res.results[c]["yt"]
        out[c * T_PER_CORE : (c + 1) * T_PER_CORE] = (
            yt.transpose(2, 0, 1).reshape(T_PER_CORE, D_OUT)
        )
    return out
